# revision 1
# baseline (speedup 1.0000x reference)
"""Trainium2 Bass kernel for AtomGCNLayer (ResGatedGraphConv + BatchNorm + ReLU + residual).

Strategy (8 NeuronCores, SPMD):
  - Host: sort edges by target node; partition the 500k nodes into 64-node
    "windows" and the windows contiguously across the 8 cores (target-parallel
    sharding => no cross-core reduction of node aggregates needed).
    Host also pre-gathers x[tgt], x[src] rows per edge (index prep only) and
    lays out everything feature-major so all device DMA is dense.
  - Device phase 1 (per core): for each 128-edge block, one bf16 matmul
    computes gate-arg g = Wk^T x_t + Wq^T x_s + We^T ea + (bk+bq) and
    v = Wv^T x_s + bv (edge-major [128,32] PSUM out).  ACT sigmoid + DVE mul
    form msg = sigmoid(g)*v.  A one-hot matrix OH[edge,node-in-window] built
    with a single DVE is_equal per group scatters messages via PE matmul
    (OH^T @ msg accumulated in PSUM per 64-node window), plus a fused skip
    matmul (x@Ws + bs + bias) per window.  BN partial stats via ones-matmul.
  - Host: reduce the 8 cores' BN partial stats -> scale/shift.
  - Device phase 2: h_norm*scale+shift, ReLU, +x residual.
"""

import math

import numpy as np
import ml_dtypes

from concourse import bacc, mybir
import concourse.tile as tile
from concourse.bass_utils import run_bass_kernel_spmd

BF16 = ml_dtypes.bfloat16

N = 500000
E = 5000000
D = 16
NC = 8
W = 64            # nodes per scatter window
PW = 1024         # windows per core
NODES_C = W * PW  # 65536 nodes per core
NPAD = NC * NODES_C
NWIN = NC * PW
NBANK = PW // 64  # PSUM agg banks per core (64 windows per bank)
BN_EPS = 1e-5

_nc_cache = {}


def _build_phase1(B, reps=1, no_scatter=False, no_oh=False, no_main=False,
                  dma_only=False, dma_mode="mix"):
    GB = 2 * B if 2 * B * 32 <= 512 else B   # blocks per group
    WPG = GB // B                            # windows per group
    ngroups = PW // WPG
    gpb = 64 // WPG                          # groups per agg bank
    SC = 8 if gpb % 8 == 0 else gpb          # groups per DMA superchunk
    nsc = ngroups // SC
    S_c = PW * B * 128                       # slots per core

    bf = mybir.dt.bfloat16
    f32 = mybir.dt.float32
    nc = bacc.Bacc(None, target_bir_lowering=False, debug=True)
    INP = nc.dram_tensor("inp", [49, S_c // 128, 128], bf, kind="ExternalInput")
    TREL = nc.dram_tensor("trel", [nsc, 128, SC, GB], bf, kind="ExternalInput")
    IOTA = nc.dram_tensor("iota", [128, W, GB], bf, kind="ExternalInput")
    WGT = nc.dram_tensor("wgt", [49, 32], bf, kind="ExternalInput")
    XSK = nc.dram_tensor("xsk", [17, NODES_C], f32, kind="ExternalInput")
    WSB = nc.dram_tensor("wsb", [17, 16], f32, kind="ExternalInput")
    H = nc.dram_tensor("h", [NBANK, 128, 32, 16], f32, kind="ExternalOutput")
    STATS = nc.dram_tensor("stats", [1, 1024], f32, kind="ExternalOutput")

    SIG = mybir.ActivationFunctionType.Sigmoid
    MUL = mybir.AluOpType.mult
    EQ = mybir.AluOpType.is_equal

    with tile.TileContext(nc) as tc:
        with (
            tc.tile_pool(name="const", bufs=1) as cpool,
            tc.tile_pool(name="sbuf", bufs=3) as pool,
            tc.tile_pool(name="xskp", bufs=2) as xpool,
            tc.tile_pool(name="pm", bufs=2, space="PSUM") as pm,
            tc.tile_pool(name="pa", bufs=2, space="PSUM") as pa,
            tc.tile_pool(name="pst", bufs=1, space="PSUM") as pst,
        ):
            wt = cpool.tile([49, 32], bf)
            nc.sync.dma_start(wt[:], WGT[:])
            wsb = cpool.tile([17, 16], f32)
            nc.sync.dma_start(wsb[:], WSB[:])
            it = cpool.tile([128, W, GB], bf)
            nc.sync.dma_start(it[:], IOTA[:])
            ones = cpool.tile([128, 1], f32)
            nc.gpsimd.memset(ones[:], 1.0)

            ssum = pst.tile([1, 512], f32, space="PSUM", tag="ssum")
            ssq = pst.tile([1, 512], f32, space="PSUM", tag="ssq")

            for rep in range(reps):
              for k in range(NBANK):
                agg = pa.tile([128, 32, 16], f32, space="PSUM", tag="agg")
                xsk = xpool.tile([17, 64, W], f32, tag="xsk")
                nc.scalar.dma_start(xsk[:], XSK[:, k * 4096:(k + 1) * 4096])
                for ss in range(gpb // SC):
                    sc = k * (gpb // SC) + ss
                    ic = pool.tile([49, SC * GB, 128], bf, tag="ic")
                    ic_eng = nc.sync if (dma_mode == "sync" or ss % 2 == 0) else nc.gpsimd
                    ic_eng.dma_start(ic[:], INP[:, sc * SC * GB:(sc + 1) * SC * GB, :])
                    tct8 = pool.tile([128, SC, GB], bf, tag="tct")
                    nc.scalar.dma_start(tct8[:], TREL[sc])
                    if dma_only:
                        continue
                    for gg2 in range(SC):
                      gg = ss * SC + gg2
                      if True:
                        mm = pm.tile([128, GB, 32], f32, space="PSUM", tag="mm")
                        if not no_main:
                            for b in range(GB):
                                nc.tensor.matmul(mm[:, b, :],
                                                 lhsT=ic[:, gg2 * GB + b, :],
                                                 rhs=wt[:], start=True, stop=True)
                        sg = pool.tile([128, GB, 16], bf, tag="sg")
                        nc.scalar.activation(sg[:], mm[:, :, 0:16], func=SIG)
                        msg = pool.tile([128, GB, 16], bf, tag="msg")
                        nc.vector.tensor_tensor(msg[:], sg[:], mm[:, :, 16:32], op=MUL)
                        oh = pool.tile([128, W, GB], bf, tag="oh")
                        if no_oh:
                            nc.gpsimd.memset(oh[:], 0.0)
                        else:
                            nc.vector.tensor_tensor(
                                oh[:],
                                tct8[:, gg2, :].unsqueeze(1).to_broadcast([128, W, GB]),
                                it[:],
                                op=EQ,
                            )
                        if no_scatter:
                            continue
                        for wi in range(WPG):
                            win = gg * WPG + wi
                            pos = 64 * (win % 2)
                            col = win // 2
                            out_ap = agg[pos:pos + 64, col, :]
                            for b in range(B):
                                blk = wi * B + b
                                nc.tensor.matmul(out_ap, lhsT=oh[:, :, blk],
                                                 rhs=msg[:, blk, :],
                                                 start=(b == 0), stop=False,
                                                 tile_position=(0, pos))
                            nc.tensor.matmul(out_ap, lhsT=xsk[:, win, :], rhs=wsb[:],
                                             start=False, stop=True,
                                             tile_position=(0, pos))
                hsb = pool.tile([128, 32, 16], f32, tag="hsb")
                if not (no_scatter or dma_only):
                    nc.vector.tensor_copy(hsb[:], agg[:])
                else:
                    nc.gpsimd.memset(hsb[:], 0.0)
                nc.sync.dma_start(H[k], hsb[:])
                if dma_only:
                    continue
                hsq = pool.tile([128, 32, 16], f32, tag="hsq")
                nc.vector.tensor_tensor(hsq[:], hsb[:], hsb[:], op=MUL)
                nc.tensor.matmul(ssum[:], lhsT=ones[:], rhs=hsb[:],
                                 start=(k == 0), stop=(k == NBANK - 1),
                                 skip_group_check=True)
                nc.tensor.matmul(ssq[:], lhsT=ones[:], rhs=hsq[:],
                                 start=(k == 0), stop=(k == NBANK - 1),
                                 skip_group_check=True)
            stsb = pool.tile([1, 1024], f32, tag="stsb")
            nc.vector.tensor_copy(stsb[:, 0:512], ssum[:])
            nc.vector.tensor_copy(stsb[:, 512:1024], ssq[:])
            nc.sync.dma_start(STATS[:], stsb[:])
    nc.compile()
    return nc


def _build_phase2():
    f32 = mybir.dt.float32
    nc = bacc.Bacc(None, target_bir_lowering=False, debug=True)
    H = nc.dram_tensor("h", [NBANK, 128, 32, 16], f32, kind="ExternalInput")
    X = nc.dram_tensor("x", [NBANK, 128, 32, 16], f32, kind="ExternalInput")
    SCL = nc.dram_tensor("scl", [128, 16], f32, kind="ExternalInput")
    SFT = nc.dram_tensor("sft", [128, 16], f32, kind="ExternalInput")
    Y = nc.dram_tensor("y", [NBANK, 128, 32, 16], f32, kind="ExternalOutput")
    ADD = mybir.AluOpType.add
    MUL = mybir.AluOpType.mult
    MAX = mybir.AluOpType.max
    with tile.TileContext(nc) as tc:
        with (
            tc.tile_pool(name="const", bufs=1) as cpool,
            tc.tile_pool(name="sbuf", bufs=3) as pool,
        ):
            scl = cpool.tile([128, 16], f32)
            nc.sync.dma_start(scl[:], SCL[:])
            sft = cpool.tile([128, 16], f32)
            nc.sync.dma_start(sft[:], SFT[:])
            scl_b = scl[:].unsqueeze(1).to_broadcast([128, 32, 16])
            sft_b = sft[:].unsqueeze(1).to_broadcast([128, 32, 16])
            for k in range(NBANK):
                h = pool.tile([128, 32, 16], f32, tag="h")
                nc.sync.dma_start(h[:], H[k])
                xb = pool.tile([128, 32, 16], f32, tag="xb")
                nc.sync.dma_start(xb[:], X[k])
                t1 = pool.tile([128, 32, 16], f32, tag="t1")
                nc.vector.tensor_tensor(t1[:], h[:], scl_b, op=MUL)
                nc.vector.tensor_tensor(t1[:], t1[:], sft_b, op=ADD)
                nc.vector.tensor_scalar(t1[:], t1[:], 0.0, None, op0=MAX)
                nc.vector.tensor_tensor(t1[:], t1[:], xb[:], op=ADD)
                yb = pool.tile([128, 32, 16], f32, tag="yb")
                nc.vector.tensor_copy(yb[:], t1[:])
                nc.sync.dma_start(Y[k], yb[:])
    nc.compile()
    return nc


def host_prep(x, edge_index, edge_attr):
    """Build all per-core device arrays. Index math + layout only."""
    src = np.asarray(edge_index[0], dtype=np.int64)
    tgt = np.asarray(edge_index[1], dtype=np.int64)
    x = np.asarray(x, dtype=np.float32)
    ea = np.asarray(edge_attr, dtype=np.float32)

    perm = np.argsort(tgt, kind="stable")
    tgt_s = tgt[perm]
    src_s = src[perm]
    wid = tgt_s // W
    counts = np.bincount(wid, minlength=NWIN)
    B = max(1, int(math.ceil(counts.max() / 128)))
    S_w = 128 * B
    S = NWIN * S_w
    S_c = PW * S_w
    starts = np.zeros(NWIN + 1, np.int64)
    starts[1:] = np.cumsum(counts)
    slots = wid * S_w + (np.arange(E, dtype=np.int64) - starts[wid])

    GB = 2 * B if 2 * B * 32 <= 512 else B
    ngroups = PW // (GB // B)
    gpb = 64 // (GB // B)
    SC = 8 if gpb % 8 == 0 else gpb
    nsc = ngroups // SC

    x16 = x.astype(BF16)
    pay = np.zeros((S, 48), BF16)
    pay[slots, 0:16] = x16[tgt_s]
    pay[slots, 16:32] = x16[src_s]
    pay[slots, 32:48] = ea[perm].astype(BF16)

    trel = np.full(S, -1.0, np.float32)
    trel[slots] = (tgt_s % W).astype(np.float32)
    trel16 = trel.astype(BF16)

    xpad = np.zeros((NPAD, D), np.float32)
    xpad[:N] = x
    mask = np.zeros(NPAD, np.float32)
    mask[:N] = 1.0

    iota = np.broadcast_to(
        np.repeat(np.arange(W, dtype=np.float32), GB).astype(BF16).reshape(1, W, GB),
        (128, W, GB)).copy()

    in_maps = []
    for c in range(NC):
        inp_c = np.empty((49, S_c), BF16)
        inp_c[0:48] = pay[c * S_c:(c + 1) * S_c].T
        inp_c[48] = BF16(1.0)
        inp_c = inp_c.reshape(49, S_c // 128, 128)
        trel_c = (trel16[c * S_c:(c + 1) * S_c]
                  .reshape(nsc, SC, GB, 128).transpose(0, 3, 1, 2).copy())
        xsk_c = np.empty((17, NODES_C), np.float32)
        xsk_c[0:16] = xpad[c * NODES_C:(c + 1) * NODES_C].T
        xsk_c[16] = mask[c * NODES_C:(c + 1) * NODES_C]
        in_maps.append({
            "inp": inp_c, "trel": trel_c, "iota": iota,
            "xsk": xsk_c,
        })
    return B, in_maps, xpad


def weight_arrays(Wk, bk, Wq, bq, Wv, bv, We, Ws, bs, bias):
    wgt = np.zeros((49, 32), np.float32)
    wgt[0:16, 0:16] = Wk
    wgt[16:32, 0:16] = Wq
    wgt[32:48, 0:16] = We
    wgt[48, 0:16] = bk + bq
    wgt[16:32, 16:32] = Wv
    wgt[48, 16:32] = bv
    wsb = np.zeros((17, 16), np.float32)
    wsb[0:16] = Ws
    wsb[16] = bs + bias
    return wgt.astype(BF16), wsb


def x_tiled(xpad):
    # [NC, NBANK, 128, 32, 16]; node n in core c: n = k*4096 + col*128 + p
    xt = xpad.reshape(NC, NBANK, 32, 128, D).transpose(0, 1, 3, 2, 4).copy()
    return xt


def untile_y(y_t):
    # y_t: [NC, NBANK, 128, 32, 16] -> [NPAD, 16]
    return y_t.transpose(0, 1, 3, 2, 4).reshape(NPAD, D)


def kernel(**inputs):
    x = np.asarray(inputs["x"], np.float32)
    B, in_maps, xpad = host_prep(x, inputs["edge_index"], inputs["edge_attr"])
    wgt, wsb = weight_arrays(
        np.asarray(inputs["Wk"], np.float32), np.asarray(inputs["bk"], np.float32),
        np.asarray(inputs["Wq"], np.float32), np.asarray(inputs["bq"], np.float32),
        np.asarray(inputs["Wv"], np.float32), np.asarray(inputs["bv"], np.float32),
        np.asarray(inputs["We"], np.float32), np.asarray(inputs["Ws"], np.float32),
        np.asarray(inputs["bs"], np.float32), np.asarray(inputs["bias"], np.float32))
    for m in in_maps:
        m["wgt"] = wgt
        m["wsb"] = wsb

    if ("p1", B) not in _nc_cache:
        _nc_cache[("p1", B)] = _build_phase1(B)
    nc1 = _nc_cache[("p1", B)]
    res1 = run_bass_kernel_spmd(nc1, in_maps, list(range(NC)))

    # host BN stats reduction
    tot = np.zeros(2, np.float64)
    s_sum = np.zeros(D, np.float64)
    s_sq = np.zeros(D, np.float64)
    for c in range(NC):
        st = res1.results[c]["stats"].astype(np.float64).reshape(2, 32, D)
        s_sum += st[0].sum(axis=0)
        s_sq += st[1].sum(axis=0)
    mean = s_sum / N
    var = s_sq / N - mean ** 2
    gamma = np.asarray(inputs["gamma"], np.float32).astype(np.float64)
    beta = np.asarray(inputs["beta"], np.float32).astype(np.float64)
    scale = gamma / np.sqrt(var + BN_EPS)
    shift = beta - mean * scale
    scl = np.broadcast_to(scale.astype(np.float32), (128, D)).copy()
    sft = np.broadcast_to(shift.astype(np.float32), (128, D)).copy()

    xt = x_tiled(xpad)
    in_maps2 = [{
        "h": res1.results[c]["h"],
        "x": xt[c],
        "scl": scl,
        "sft": sft,
    } for c in range(NC)]
    if "p2" not in _nc_cache:
        _nc_cache["p2"] = _build_phase2()
    nc2 = _nc_cache["p2"]
    res2 = run_bass_kernel_spmd(nc2, in_maps2, list(range(NC)))

    y_t = np.stack([res2.results[c]["y"] for c in range(NC)])
    y = untile_y(y_t)[:N]
    return y.astype(np.float32)



# revision 4
# speedup vs baseline: 1.9170x; 1.9170x over previous
"""Trainium2 Bass kernel for AtomGCNLayer (ResGatedGraphConv + BatchNorm + ReLU + residual).

v2 strategy (8 NeuronCores, SPMD), evolved from the one-hot-scatter baseline:
  - Host: degree-balanced node->window packing. Nodes are sorted by in-degree
    and dealt round-robin into 8192 windows of <=64 nodes, so every window has
    nearly equal edge count (~610) and the per-window slot allocation drops
    from B=6 (768 slots) to B=5 (640): ~17% less DMA + per-slot engine work.
  - Device phase 1 (per core): per 128-edge block one bf16 matmul computes
    gate-arg and v ([128,32] PSUM). ACT sigmoid -> sg (SBUF bf16); gpsimd
    (Pool engine) does msg = sg * v, freeing the DVE; DVE only builds the
    one-hot scatter matrix (merged 2 groups per instruction). PE scatters
    msg via OH^T matmuls + fused skip matmul per 64-node window. BN partial
    stats via ones-matmuls; bank tails (copy/square) on ACT.
  - Host: reduce the 8 cores' BN partial stats -> scale/shift.
  - Device phase 2: h*scale+shift, ReLU, +x residual (engine-balanced).
"""

import math

import numpy as np
import ml_dtypes

from concourse import bacc, mybir
import concourse.tile as tile
from concourse.bass_utils import run_bass_kernel_spmd

BF16 = ml_dtypes.bfloat16

N = 500000
E = 5000000
D = 16
NC = 8
W = 64            # nodes per scatter window
PW = 1024         # windows per core
NODES_C = W * PW  # 65536 nodes per core
NPAD = NC * NODES_C
NWIN = NC * PW
NBANK = PW // 64  # 16 agg banks per core (64 windows per bank)
BN_EPS = 1e-5

_nc_cache = {}
_gslot = None     # node -> global padded slot, set by host_prep


def _build_phase1(B):
    GB = 2 * B            # blocks per group (2 windows per group)
    SC = 8                # groups per DMA superchunk
    gpb = 32              # groups per agg bank
    nsc_pb = gpb // SC    # superchunks per bank
    nsc = NBANK * nsc_pb
    S_c = PW * B * 128

    bf = mybir.dt.bfloat16
    f32 = mybir.dt.float32
    nc = bacc.Bacc(None, target_bir_lowering=False, debug=True)
    INP = nc.dram_tensor("inp", [49, S_c // 128, 128], bf, kind="ExternalInput")
    TREL = nc.dram_tensor("trel", [nsc, 128, SC, GB], bf, kind="ExternalInput")
    IOTA = nc.dram_tensor("iota", [128, W, 2, GB], bf, kind="ExternalInput")
    WGT = nc.dram_tensor("wgt", [49, 32], bf, kind="ExternalInput")
    XSK = nc.dram_tensor("xsk", [17, NODES_C], f32, kind="ExternalInput")
    WSB = nc.dram_tensor("wsb", [17, 16], f32, kind="ExternalInput")
    H = nc.dram_tensor("h", [NBANK, 128, 32, 16], f32, kind="ExternalOutput")
    STATS = nc.dram_tensor("stats", [2, 512], f32, kind="ExternalOutput")

    SIG = mybir.ActivationFunctionType.Sigmoid
    SQ = mybir.ActivationFunctionType.Square
    CP = mybir.ActivationFunctionType.Copy
    MUL = mybir.AluOpType.mult
    EQ = mybir.AluOpType.is_equal

    with tile.TileContext(nc) as tc:
        with (
            tc.tile_pool(name="const", bufs=1) as cpool,
            tc.tile_pool(name="sbuf", bufs=3) as pool,
            tc.tile_pool(name="xskp", bufs=2) as xpool,
            tc.tile_pool(name="pm", bufs=2, space="PSUM") as pm,
            tc.tile_pool(name="pa", bufs=2, space="PSUM") as pa,
            tc.tile_pool(name="pst", bufs=1, space="PSUM") as pst,
        ):
            wt = cpool.tile([49, 32], bf)
            nc.sync.dma_start(wt[:], WGT[:])
            wsb = cpool.tile([17, 16], f32)
            nc.sync.dma_start(wsb[:], WSB[:])
            it = cpool.tile([128, W, 2, GB], bf)
            nc.sync.dma_start(it[:], IOTA[:])
            ones = cpool.tile([128, 1], f32)
            nc.gpsimd.memset(ones[:], 1.0)
            onesb = cpool.tile([128, 1], bf)
            nc.gpsimd.memset(onesb[:], 1.0)

            sstat = pst.tile([33, 512], f32, space="PSUM", tag="sstat")

            for k in range(NBANK):
                agg = pa.tile([128, 32, 16], f32, space="PSUM", tag="agg")
                xsk = xpool.tile([17, 64, W], f32, tag="xsk")
                nc.sync.dma_start(xsk[:], XSK[:, k * 4096:(k + 1) * 4096])
                for ss in range(nsc_pb):
                    sc = k * nsc_pb + ss
                    ic = pool.tile([49, SC * GB, 128], bf, tag="ic")
                    ic_eng = nc.sync if ss % 2 == 0 else nc.gpsimd
                    ic_eng.dma_start(ic[:], INP[:, sc * SC * GB:(sc + 1) * SC * GB, :])
                    tct8 = pool.tile([128, SC, GB], bf, tag="tct")
                    nc.sync.dma_start(tct8[:], TREL[sc])
                    for pp in range(SC // 2):
                        mm = pm.tile([128, 2, GB, 32], f32, space="PSUM", tag="mm")
                        for i in range(2):
                            for b in range(GB):
                                nc.tensor.matmul(mm[:, i, b, :],
                                                 lhsT=ic[:, (2 * pp + i) * GB + b, :],
                                                 rhs=wt[:], start=True, stop=True)
                        sg = pool.tile([128, 2, GB, 16], bf, tag="sg")
                        nc.scalar.activation(sg[:], mm[:, :, :, 0:16], func=SIG)
                        vb = pool.tile([128, 2, GB, 16], bf, tag="vb")
                        nc.scalar.activation(vb[:], mm[:, :, :, 16:32], func=CP)
                        oh = pool.tile([128, W, 2, GB], bf, tag="oh")
                        nc.vector.tensor_tensor(
                            oh[:],
                            tct8[:, 2 * pp:2 * pp + 2, :].unsqueeze(1)
                                .to_broadcast([128, W, 2, GB]),
                            it[:],
                            op=EQ,
                        )
                        msg = pool.tile([128, 2, GB, 16], bf, tag="msg")
                        nc.gpsimd.tensor_tensor(msg[:], sg[:], vb[:], op=MUL)
                        for i in range(2):
                            gg = ss * SC + 2 * pp + i
                            for wi in range(2):
                                win_b = gg * 2 + wi
                                pos = 64 * (win_b % 2)
                                col = win_b // 2
                                out_ap = agg[pos:pos + 64, col, :]
                                for b in range(B):
                                    blk = wi * B + b
                                    nc.tensor.matmul(out_ap,
                                                     lhsT=oh[:, :, i, blk],
                                                     rhs=msg[:, i, blk, :],
                                                     start=(b == 0), stop=False,
                                                     tile_position=(0, pos))
                                nc.tensor.matmul(out_ap, lhsT=xsk[:, win_b, :],
                                                 rhs=wsb[:],
                                                 start=False, stop=True,
                                                 tile_position=(0, pos))
                hsb = pool.tile([128, 32, 16], f32, tag="hsb")
                nc.scalar.activation(hsb[:], agg[:], func=CP)
                nc.sync.dma_start(H[k], hsb[:])
                hsq = pool.tile([128, 32, 16], bf, tag="hsq")
                nc.scalar.activation(hsq[:], agg[:], func=SQ)
                nc.tensor.matmul(sstat[0:1, :], lhsT=ones[:], rhs=hsb[:],
                                 start=(k == 0), stop=(k == NBANK - 1),
                                 tile_position=(0, 0), skip_group_check=True)
                nc.tensor.matmul(sstat[32:33, :], lhsT=onesb[:], rhs=hsq[:],
                                 start=(k == 0), stop=(k == NBANK - 1),
                                 tile_position=(0, 32), skip_group_check=True)
            stsb0 = pool.tile([1, 512], f32, tag="stsb0")
            nc.vector.tensor_copy(stsb0[:], sstat[0:1, :])
            nc.sync.dma_start(STATS[0:1, :], stsb0[:])
            stsb1 = pool.tile([1, 512], f32, tag="stsb1")
            nc.vector.tensor_copy(stsb1[:], sstat[32:33, :])
            nc.sync.dma_start(STATS[1:2, :], stsb1[:])
    nc.compile()
    return nc


def _build_phase2():
    f32 = mybir.dt.float32
    bf = mybir.dt.bfloat16
    nc = bacc.Bacc(None, target_bir_lowering=False, debug=True)
    H = nc.dram_tensor("h", [NBANK, 128, 32, 16], f32, kind="ExternalInput")
    X = nc.dram_tensor("x", [NBANK, 128, 32, 16], bf, kind="ExternalInput")
    SCL = nc.dram_tensor("scl", [128, 16], f32, kind="ExternalInput")
    SFT = nc.dram_tensor("sft", [128, 16], f32, kind="ExternalInput")
    Y = nc.dram_tensor("y", [NBANK, 128, 32, 16], f32, kind="ExternalOutput")
    ADD = mybir.AluOpType.add
    MUL = mybir.AluOpType.mult
    RELU = mybir.ActivationFunctionType.Relu
    with tile.TileContext(nc) as tc:
        with (
            tc.tile_pool(name="const", bufs=1) as cpool,
            tc.tile_pool(name="sbuf", bufs=3) as pool,
        ):
            scl = cpool.tile([128, 16], f32)
            nc.sync.dma_start(scl[:], SCL[:])
            sft = cpool.tile([128, 16], f32)
            nc.sync.dma_start(sft[:], SFT[:])
            scl_b = scl[:].unsqueeze(1).to_broadcast([128, 32, 16])
            sft_b = sft[:].unsqueeze(1).to_broadcast([128, 32, 16])
            for k in range(NBANK):
                h = pool.tile([128, 32, 16], f32, tag="h")
                nc.sync.dma_start(h[:], H[k])
                xb = pool.tile([128, 32, 16], bf, tag="xb")
                nc.sync.dma_start(xb[:], X[k])
                t1 = pool.tile([128, 32, 16], f32, tag="t1")
                nc.vector.tensor_tensor(t1[:], h[:], scl_b, op=MUL)
                nc.vector.tensor_tensor(t1[:], t1[:], sft_b, op=ADD)
                t2 = pool.tile([128, 32, 16], f32, tag="t2")
                nc.scalar.activation(t2[:], t1[:], func=RELU)
                yb = pool.tile([128, 32, 16], f32, tag="yb")
                nc.gpsimd.tensor_tensor(yb[:], t2[:], xb[:], op=ADD)
                nc.sync.dma_start(Y[k], yb[:])
    nc.compile()
    return nc


def host_prep(x, edge_index, edge_attr):
    """Degree-balanced window packing + per-core device array layout."""
    global _gslot
    src = np.asarray(edge_index[0], dtype=np.int64)
    tgt = np.asarray(edge_index[1], dtype=np.int64)
    x = np.asarray(x, dtype=np.float32)
    ea = np.asarray(edge_attr, dtype=np.float32)

    # --- node -> (window, pos) by round-robin deal of degree-sorted nodes ---
    deg = np.bincount(tgt, minlength=N)
    order = np.argsort(-deg, kind="stable")
    node2win = np.empty(N, np.int64)
    node2pos = np.empty(N, np.int64)
    node2win[order] = np.arange(N, dtype=np.int64) % NWIN
    node2pos[order] = np.arange(N, dtype=np.int64) // NWIN
    _gslot = node2win * W + node2pos

    wcnt = np.zeros(NWIN, np.int64)
    np.add.at(wcnt, node2win, deg)
    B = max(1, int(math.ceil(wcnt.max() / 128)))
    S_w = 128 * B
    S = NWIN * S_w
    S_c = PW * S_w

    # --- edge slots: sort edges by target window ---
    ew = node2win[tgt]
    perm = np.argsort(ew, kind="stable")
    ew_s = ew[perm]
    starts = np.zeros(NWIN + 1, np.int64)
    starts[1:] = np.cumsum(wcnt)
    slots = ew_s * S_w + (np.arange(E, dtype=np.int64) - starts[ew_s])

    GB = 2 * B
    SC = 8
    nsc_pb = 32 // SC
    nsc = NBANK * nsc_pb

    tgt_s = tgt[perm]
    src_s = src[perm]
    x16 = x.astype(BF16)
    pay = np.zeros((S, 48), BF16)
    pay[slots, 0:16] = x16[tgt_s]
    pay[slots, 16:32] = x16[src_s]
    pay[slots, 32:48] = ea[perm].astype(BF16)

    trel = np.full(S, -1.0, np.float32)
    trel[slots] = node2pos[tgt_s].astype(np.float32)
    trel16 = trel.astype(BF16)

    # x in padded-slot order
    xpad = np.zeros((NPAD, D), np.float32)
    xpad[_gslot] = x
    mask = np.zeros(NPAD, np.float32)
    mask[_gslot] = 1.0

    iota = np.broadcast_to(
        np.arange(W, dtype=np.float32).astype(BF16).reshape(1, W, 1, 1),
        (128, W, 2, GB)).copy()

    in_maps = []
    for c in range(NC):
        inp_c = np.empty((49, S_c), BF16)
        inp_c[0:48] = pay[c * S_c:(c + 1) * S_c].T
        inp_c[48] = BF16(1.0)
        inp_c = inp_c.reshape(49, S_c // 128, 128)
        trel_c = (trel16[c * S_c:(c + 1) * S_c]
                  .reshape(nsc, SC, GB, 128).transpose(0, 3, 1, 2).copy())
        xsk_c = np.empty((17, NODES_C), np.float32)
        xsk_c[0:16] = xpad[c * NODES_C:(c + 1) * NODES_C].T
        xsk_c[16] = mask[c * NODES_C:(c + 1) * NODES_C]
        in_maps.append({
            "inp": inp_c, "trel": trel_c, "iota": iota,
            "xsk": xsk_c,
        })
    return B, in_maps, xpad


def weight_arrays(Wk, bk, Wq, bq, Wv, bv, We, Ws, bs, bias):
    wgt = np.zeros((49, 32), np.float32)
    wgt[0:16, 0:16] = Wk
    wgt[16:32, 0:16] = Wq
    wgt[32:48, 0:16] = We
    wgt[48, 0:16] = bk + bq
    wgt[16:32, 16:32] = Wv
    wgt[48, 16:32] = bv
    wsb = np.zeros((17, 16), np.float32)
    wsb[0:16] = Ws
    wsb[16] = bs + bias
    return wgt.astype(BF16), wsb


def x_tiled(xpad):
    # [NC, NBANK, 128, 32, 16]; slot s in core c: s = k*4096 + col*128 + p
    xt = xpad.reshape(NC, NBANK, 32, 128, D).transpose(0, 1, 3, 2, 4)
    return np.ascontiguousarray(xt).astype(BF16)


def untile_y(y_t):
    # y_t: [NC, NBANK, 128, 32, 16] -> [NPAD, 16]
    return y_t.transpose(0, 1, 3, 2, 4).reshape(NPAD, D)


def kernel(**inputs):
    x = np.asarray(inputs["x"], np.float32)
    B, in_maps, xpad = host_prep(x, inputs["edge_index"], inputs["edge_attr"])
    wgt, wsb = weight_arrays(
        np.asarray(inputs["Wk"], np.float32), np.asarray(inputs["bk"], np.float32),
        np.asarray(inputs["Wq"], np.float32), np.asarray(inputs["bq"], np.float32),
        np.asarray(inputs["Wv"], np.float32), np.asarray(inputs["bv"], np.float32),
        np.asarray(inputs["We"], np.float32), np.asarray(inputs["Ws"], np.float32),
        np.asarray(inputs["bs"], np.float32), np.asarray(inputs["bias"], np.float32))
    for m in in_maps:
        m["wgt"] = wgt
        m["wsb"] = wsb

    if ("p1", B) not in _nc_cache:
        _nc_cache[("p1", B)] = _build_phase1(B)
    nc1 = _nc_cache[("p1", B)]
    res1 = run_bass_kernel_spmd(nc1, in_maps, list(range(NC)))

    # host BN stats reduction
    s_sum = np.zeros(D, np.float64)
    s_sq = np.zeros(D, np.float64)
    for c in range(NC):
        st = res1.results[c]["stats"].astype(np.float64).reshape(2, 32, D)
        s_sum += st[0].sum(axis=0)
        s_sq += st[1].sum(axis=0)
    mean = s_sum / N
    var = s_sq / N - mean ** 2
    gamma = np.asarray(inputs["gamma"], np.float32).astype(np.float64)
    beta = np.asarray(inputs["beta"], np.float32).astype(np.float64)
    scale = gamma / np.sqrt(var + BN_EPS)
    shift = beta - mean * scale
    scl = np.broadcast_to(scale.astype(np.float32), (128, D)).copy()
    sft = np.broadcast_to(shift.astype(np.float32), (128, D)).copy()

    xt = x_tiled(xpad)
    in_maps2 = [{
        "h": res1.results[c]["h"],
        "x": xt[c],
        "scl": scl,
        "sft": sft,
    } for c in range(NC)]
    if "p2" not in _nc_cache:
        _nc_cache["p2"] = _build_phase2()
    nc2 = _nc_cache["p2"]
    res2 = run_bass_kernel_spmd(nc2, in_maps2, list(range(NC)))

    y_t = np.stack([res2.results[c]["y"] for c in range(NC)])
    y = untile_y(y_t)[_gslot]
    return y.astype(np.float32)
